# revision 1
# baseline (speedup 1.0000x reference)
"""Trainium2 Bass kernel: per-pixel 5x5 kernel application (KPN-style).

    out[b,c,y,x] = sum_{i,j} softmax(kernels[b,:,y,x])[i*5+j]
                   * zpad(data)[b,c,y+i,x+j]          (i,j in 0..4, r=2)

Sharding (8 NeuronCores, pure data parallel, no collectives):
    core = (b, H-half): 4 batches x 2 row-bands of 360 rows.
    Halo rows come from overlapping host-side slices of the full input.

Per-core algorithm (tiles live in "data space": 124 partitions =
120 output rows + 2 halo rows each side):
    - unnormalized softmax: E_t = exp(K_t) on ACT (bf16), denominator
      accumulated on the PE together with the taps.
    - DVE forms per-tap product planes Q_{t,c} = E_t * D_c (bf16, 2x mode);
      x-shift dj is a free-dim offset (two parity-aligned bf16 copies of the
      data keep operands 4-byte aligned); y-shift di is folded into the
      kernel-tensor DMA (rows loaded shifted by -di) and undone by the PE's
      stationary shift matrix S_di[k,m] = [k == m+di].
    - PE accumulates the 25 tap planes (and the 25 exp planes) into PSUM
      with shifted-identity matmuls.
    - DVE: out_c = PSUM_c * reciprocal(PSUM_sum).

DMA layout notes: kernel-tensor loads are one DMA per tap plane so the
DRAM side is a single contiguous region (strided small-chunk patterns get
pinned to a few SDMA engines); inputs are pre-converted to bf16 on the
host to halve HBM traffic; loads alternate between the two HWDGE rings
(sync + scalar); the output is staged full-width and stored once per
row-tile.

kernel(**inputs) takes the FULL inputs and returns the FULL output.
"""

import numpy as np
import ml_dtypes

B, C, H, W, KW = 4, 3, 720, 1280, 5
NCORES = 8
HS = H // 2            # 360 output rows per shard
RT = 120               # output rows per row-tile
NRT = HS // RT         # 3 row-tiles
HALO = 2
DP = RT + 2 * HALO     # 124 partitions (data space)
WP = 1288              # padded data width: 2 left + 1280 + 6 right
KROWPAD = 4            # zero rows around each kernel shard (top+bottom)
KH = HS + 2 * KROWPAD  # 368
XCH = [(0, 512), (512, 512), (1024, 256)]

KERN_BF16 = True       # ship kernels to HBM as bf16 (halves DMA traffic)

_CACHE = {}


def _build_program():
    import concourse.bacc as bacc
    import concourse.mybir as mybir
    from concourse.bass import AP
    from concourse import tile

    f32 = mybir.dt.float32
    bf16 = mybir.dt.bfloat16
    kdt = bf16 if KERN_BF16 else f32

    nc = bacc.Bacc(
        "TRN2",
        target_bir_lowering=False,
        debug=False,
        enable_asserts=False,
        num_devices=NCORES,
    )
    # Row-major host layouts: one large contiguous DMA descriptor per
    # SBUF partition (small strided descriptors throttle the SDMA engines).
    d_data = nc.dram_tensor("data", [HS + 2 * HALO, C, WP], bf16, kind="ExternalInput")
    d_kern = nc.dram_tensor("kern", [KH, KW * KW, W], kdt, kind="ExternalInput")
    d_out = nc.dram_tensor("out", [HS, C, W], f32, kind="ExternalOutput")

    # Shift matrices S_di[k, m] = 1 iff k == m + di  (k: 124 data rows,
    # m: 120 out rows). Baked into the NEFF as a Const tensor.
    s_np = np.zeros((KW, DP, RT), dtype=ml_dtypes.bfloat16)
    for di in range(KW):
        for m in range(RT):
            s_np[di, m + di, m] = 1.0
    d_s = nc.inline_tensor(np.ascontiguousarray(s_np), "smat")

    KROW = KW * KW * W  # element stride between rows of d_kern

    with tile.TileContext(nc) as tc:
        with tc.tile_pool(name="const", bufs=1) as cpool, \
             tc.tile_pool(name="dbf", bufs=2) as dbfpool, \
             tc.tile_pool(name="kt", bufs=3) as kpool, \
             tc.tile_pool(name="et", bufs=7) as epool, \
             tc.tile_pool(name="qt", bufs=4) as qpool, \
             tc.tile_pool(name="fin", bufs=2) as fpool, \
             tc.tile_pool(name="ps", bufs=2, space="PSUM") as ppool:

            s_sb = cpool.tile([DP, KW, RT], bf16)
            nc.sync.dma_start(out=s_sb[:], in_=d_s.ap().transpose([1, 0, 2]))

            pending_store = []

            def flush_store():
                while pending_store:
                    yy, t = pending_store.pop()
                    nc.gpsimd.dma_start(out=d_out.ap()[yy:yy + RT], in_=t[:])

            for rt in range(NRT):
                y0 = rt * RT

                # data rows y0 .. y0+123 of the (row-padded) shard, bf16.
                # dbf1 is shifted one element left so odd-dj slices stay
                # 4-byte aligned (keeps DVE 2x mode).
                # kernel taps: one SWDGE DMA per di-group (5 tap planes,
                # rows shifted by -di) — SWDGE spreads descriptors across
                # all 16 SDMA engines (HWDGE pins them to 4); exp per group.
                # dbf0 also rides SWDGE (issued after the first kt so the
                # first exp is gated only by a 124-descriptor load); dbf1
                # (the 1-element x-shifted copy for odd-dj alignment) is a
                # cheap DVE bf16 copy, not a DMA, so no DMA->DMA dep chain.
                dbf0 = dbfpool.tile([DP, C, WP], bf16, tag="dbf0")
                dbf1 = dbfpool.tile([DP, C, WP], bf16, tag="dbf1")
                # byte-balance the three DMA paths (SWDGE sustains ~70GB/s
                # of descriptor flow; the two HWDGE rings share SDMA engines
                # 0-3 for ~110GB/s combined): di 0,1 + data on SWDGE,
                # di 2,3 on the sync ring, di 4 (+ the store) on scalar.
                ets = []
                kt_eng = [nc.sync, nc.scalar, nc.sync, nc.scalar, nc.gpsimd]
                for di in range(KW):
                    kt = kpool.tile([DP, KW, W], kdt, tag="kt")
                    et = epool.tile([DP, KW, W], bf16, tag="et")
                    for dj in range(KW):
                        off = (KROWPAD + y0 - di) * KROW + (di * KW + dj) * W
                        kt_eng[di].dma_start(
                            out=kt[:, dj, :],
                            in_=AP(d_kern, off, [[KROW, DP], [1, W]]),
                        )
                        nc.scalar.activation(
                            et[:, dj, :], kt[:, dj, :],
                            mybir.ActivationFunctionType.Exp,
                        )
                    if di == 0:
                        nc.gpsimd.dma_start(
                            out=dbf0[:], in_=d_data.ap()[y0:y0 + DP],
                        )
                    ets.append(et)
                f0 = dbf0[:].rearrange("p c w -> p (c w)")
                f1 = dbf1[:].rearrange("p c w -> p (c w)")
                nc.vector.tensor_copy(f1[:, 0:C * WP - 1], f0[:, 1:C * WP])
                flush_store()

                rs = fpool.tile([RT, W], f32, tag="rs", bufs=1)
                ost = fpool.tile([RT, C, W], f32, tag="ost")

                for (xc, xcw) in XCH:
                    # PSUM banks: 0..2 = channel accumulators, 3 = sumexp
                    pacc = ppool.tile([RT, 4, 512], f32, tag="pacc")

                    for di in range(KW):
                        et = ets[di]
                        lhs = s_sb[:, di, :]
                        first = di == 0
                        last = di == KW - 1
                        for dj in range(KW):
                            nc.tensor.matmul(
                                out=pacc[:, 3, 0:xcw],
                                lhsT=lhs,
                                rhs=et[:, dj, xc:xc + xcw],
                                start=first and dj == 0,
                                stop=last and dj == KW - 1,
                            )
                        for dj in range(KW):
                            qt = qpool.tile([DP, C, 512], bf16, tag="qt")
                            if dj % 2 == 0:
                                dsrc = dbf0[:, :, xc + dj:xc + dj + xcw]
                            else:
                                dsrc = dbf1[:, :, xc + dj - 1:xc + dj - 1 + xcw]
                            esrc = (
                                et[:, dj, xc:xc + xcw]
                                .unsqueeze(1)
                                .broadcast_to([DP, C, xcw])
                            )
                            nc.vector.tensor_tensor(
                                qt[:, :, 0:xcw], esrc, dsrc, mybir.AluOpType.mult,
                            )
                            for c in range(C):
                                nc.tensor.matmul(
                                    out=pacc[:, c, 0:xcw],
                                    lhsT=lhs,
                                    rhs=qt[:, c, 0:xcw],
                                    start=first and dj == 0,
                                    stop=last and dj == KW - 1,
                                )

                    nc.vector.reciprocal(rs[:, xc:xc + xcw], pacc[:, 3, 0:xcw])
                    rsb = (
                        rs[:, xc:xc + xcw].unsqueeze(1).broadcast_to([RT, C, xcw])
                    )
                    nc.vector.tensor_tensor(
                        ost[:, :, xc:xc + xcw], pacc[:, 0:3, 0:xcw], rsb,
                        mybir.AluOpType.mult,
                    )

                pending_store.append((y0, ost))

            flush_store()

    nc.compile()
    return nc


def get_program():
    if "nc" not in _CACHE:
        _CACHE["nc"] = _build_program()
    return _CACHE["nc"]


def make_shards(data: np.ndarray, kernels: np.ndarray):
    """Full inputs -> per-core input maps (with halo + zero padding)."""
    data = np.asarray(data, dtype=np.float32)
    kernels = np.asarray(kernels, dtype=np.float32)
    kdt = ml_dtypes.bfloat16 if KERN_BF16 else np.float32
    # zero-pad data: 2 rows top/bottom, 2 cols left, 6 cols right;
    # row-major layouts: data [row, c, x], kern [row, tap, x]
    dpad = np.zeros((B, H + 2 * HALO, C, WP), dtype=ml_dtypes.bfloat16)
    dpad[:, HALO:HALO + H, :, HALO:HALO + W] = (
        data.transpose(0, 2, 1, 3).astype(ml_dtypes.bfloat16)
    )
    in_maps = []
    for core in range(NCORES):
        b, hh = divmod(core, 2)
        r0 = hh * HS
        dsh = np.ascontiguousarray(dpad[b, r0:r0 + HS + 2 * HALO])
        ksh = np.zeros((KH, KW * KW, W), dtype=kdt)
        ksh[KROWPAD:KROWPAD + HS] = (
            kernels[b, :, r0:r0 + HS, :].transpose(1, 0, 2).astype(kdt)
        )
        in_maps.append({"data": dsh, "kern": ksh})
    return in_maps


def assemble(results) -> np.ndarray:
    out = np.empty((B, C, H, W), dtype=np.float32)
    for core in range(NCORES):
        b, hh = divmod(core, 2)
        out[b, :, hh * HS:(hh + 1) * HS, :] = results[core]["out"].transpose(1, 0, 2)
    return out


def kernel(data: np.ndarray, kernels: np.ndarray) -> np.ndarray:
    from concourse.bass_utils import run_bass_kernel_spmd

    nc = get_program()
    in_maps = make_shards(data, kernels)
    res = run_bass_kernel_spmd(nc, in_maps, list(range(NCORES)))
    return assemble(res.results)


if __name__ == "__main__":
    get_program()
    print("program built OK")



# revision 2
# speedup vs baseline: 1.4390x; 1.4390x over previous
"""Trainium2 Bass kernel: per-pixel 5x5 kernel application (KPN-style).

    out[b,c,y,x] = sum_{i,j} softmax(kernels[b,:,y,x])[i*5+j]
                   * zpad(data)[b,c,y+i,x+j]          (i,j in 0..4, r=2)

Sharding (8 NeuronCores, pure data parallel, no collectives):
    core = (b, H-half): 4 batches x 2 row-bands of 360 rows.

Band layout (v2): partition p = x-band of 10 columns (128 bands x 10 =
1280).  Rows live in the free dimension, so BOTH the di (row) and dj
(col) tap shifts become free-dim AP offsets -- no shift matrices, no
partition-crossing.  Per 45-row accumulation tile (8 per core):

    - ACT: E = exp(logits) in one [128, 11250] op (fp16).
    - DVE: per (di, c): two batched products q = E * D (dj in the AP's
      outer free dim; even dj read D0, odd dj read D1 = D0 shifted one
      element so operands stay 4-byte aligned for the 2x DVE mode).
    - PE:  identity-lhsT matmuls accumulate the 75 tap planes and the
      25 exp planes into 4 PSUM banks (start/stop per bank).  The
      stationary operand never changes, so the PE stays warm.
    - DVE: R = reciprocal_approx_fast(sumexp);  out = P * R  (fp16).

DMA: kernel-tensor loads are one SWDGE dma_start per tile with a single
22.5KB contiguous run per partition (128 descriptors, sprayed over all
16 SDMA engines).  D0/D1 ride the two HWDGE rings; stores alternate.
Inputs/outputs are fp16 (host casts); exp/normalize run on-device.

kernel(**inputs) takes the FULL inputs and returns the FULL output.
"""

import numpy as np
from numpy.lib.stride_tricks import sliding_window_view

B, C, H, W, KW = 4, 3, 720, 1280, 5
NCORES = 8
HS = H // 2            # 360 output rows per shard
NB = 128               # x-bands (partitions)
BW = 10                # band width (output columns per partition)
DW = 14                # data band width incl. 2+2 halo columns
TR = 45                # rows per accumulation tile (PSUM: 450 <= 512)
NT = HS // TR          # 8 tiles
DR = HS + 4            # 364 data rows incl. 2+2 halo rows
TAPS = KW * KW

# host tap order: within each di group, dj = 0,2,4,1,3 (even-first so
# the even/odd product APs are plain slices)
DJ_ORDER = [0, 2, 4, 1, 3]
TAP_PERM = [di * KW + dj for di in range(KW) for dj in DJ_ORDER]

_CACHE = {}


def _build_program():
    import concourse.bacc as bacc
    import concourse.mybir as mybir
    from concourse.bass import AP
    from concourse import tile

    f32 = mybir.dt.float32
    f16 = mybir.dt.float16

    nc = bacc.Bacc(
        "TRN2",
        target_bir_lowering=False,
        debug=False,
        enable_asserts=False,
        num_devices=NCORES,
    )
    d_ke = nc.dram_tensor("ke", [NB, HS, TAPS * BW], f16, kind="ExternalInput")
    d_db0 = nc.dram_tensor("db0", [NB, C, DR, DW], f16, kind="ExternalInput")
    d_db1 = nc.dram_tensor("db1", [NB, C, DR, DW], f16, kind="ExternalInput")
    d_out = nc.dram_tensor("out", [NB, HS, C * BW], f16, kind="ExternalOutput")

    d_id = nc.inline_tensor(np.eye(NB, dtype=np.float16), "ident")

    KE_ROW = TAPS * BW          # 250 elems per (band, row)
    D_CH = DR * DW              # 5096 elems per (band, channel)

    with tile.TileContext(nc) as tc:
        with tc.tile_pool(name="const", bufs=1) as cpool, \
             tc.tile_pool(name="kt", bufs=2) as kpool, \
             tc.tile_pool(name="et", bufs=2) as epool, \
             tc.tile_pool(name="dt", bufs=1) as dpool, \
             tc.tile_pool(name="qt", bufs=4) as qpool, \
             tc.tile_pool(name="rt", bufs=2) as rpool, \
             tc.tile_pool(name="ot", bufs=2) as opool, \
             tc.tile_pool(name="ps", bufs=2, space="PSUM") as ppool:

            id_sb = cpool.tile([NB, NB], f16)
            nc.sync.dma_start(out=id_sb[:], in_=d_id.ap())

            D0 = dpool.tile([NB, C, DR, DW], f16, tag="d0")
            D1 = dpool.tile([NB, C, DR, DW], f16, tag="d1")
            nc.sync.dma_start(out=D0[:], in_=d_db0.ap())
            nc.scalar.dma_start(out=D1[:], in_=d_db1.ap())

            st_eng = [nc.sync, nc.scalar]

            for t in range(NT):
                kE = kpool.tile([NB, TR, KE_ROW], f16, tag="ke")
                nc.gpsimd.dma_start(
                    out=kE[:], in_=d_ke.ap()[:, t * TR:(t + 1) * TR]
                )
                E = epool.tile([NB, TR, TAPS, BW], f16, tag="e")
                nc.scalar.activation(
                    E[:].rearrange("p r t x -> p (r t x)"),
                    kE[:].rearrange("p r k -> p (r k)"),
                    mybir.ActivationFunctionType.Exp,
                )
                eap = E[:]

                pacc = ppool.tile([NB, 4, 512], f32, tag="pacc")

                for di in range(KW):
                    # products: q = E * D, dj batched in the outer free dim
                    qes, qos = [], []
                    for c in range(C):
                        qe = qpool.tile([NB, 3, TR, BW], f16, tag="qe")
                        qo = qpool.tile([NB, 2, TR, BW], f16, tag="qo")
                        e_even = AP(
                            eap.tensor, (KW * di) * BW,
                            [[TR * KE_ROW, NB], [BW, 3], [KE_ROW, TR], [1, BW]],
                        )
                        e_odd = AP(
                            eap.tensor, (KW * di + 3) * BW,
                            [[TR * KE_ROW, NB], [BW, 2], [KE_ROW, TR], [1, BW]],
                        )
                        doff = c * D_CH + (t * TR + di) * DW
                        d_even = AP(
                            D0[:].tensor, doff,
                            [[C * D_CH, NB], [2, 3], [DW, TR], [1, BW]],
                        )
                        d_odd = AP(
                            D1[:].tensor, doff,
                            [[C * D_CH, NB], [2, 2], [DW, TR], [1, BW]],
                        )
                        nc.vector.tensor_tensor(
                            qe[:], e_even, d_even, mybir.AluOpType.mult)
                        nc.vector.tensor_tensor(
                            qo[:], e_odd, d_odd, mybir.AluOpType.mult)
                        qes.append(qe)
                        qos.append(qo)
                    # sumexp: 5 identity matmuls straight off E (no DVE dep)
                    for k in range(KW):
                        tp = KW * di + k
                        nc.tensor.matmul(
                            out=pacc[:, 3, 0:TR * BW],
                            lhsT=id_sb[:],
                            rhs=eap[:, :, tp, :],
                            start=(tp == 0),
                            stop=(tp == TAPS - 1),
                        )
                    # tap accumulation
                    for c in range(C):
                        for k in range(3):
                            nc.tensor.matmul(
                                out=pacc[:, c, 0:TR * BW],
                                lhsT=id_sb[:],
                                rhs=qes[c][:, k],
                                start=(di == 0 and k == 0),
                                stop=False,
                            )
                        for k in range(2):
                            nc.tensor.matmul(
                                out=pacc[:, c, 0:TR * BW],
                                lhsT=id_sb[:],
                                rhs=qos[c][:, k],
                                start=False,
                                stop=(di == KW - 1 and k == 1),
                            )

                R = rpool.tile([NB, TR, BW], f32, tag="r")
                nc.vector.reciprocal_approx_fast(
                    out=R[:].rearrange("p r x -> p (r x)"),
                    in_=pacc[:, 3, 0:TR * BW],
                )
                outst = opool.tile([NB, TR, C, BW], f16, tag="o")
                p_view = AP(
                    pacc[:].tensor, 0,
                    [[4 * 512, NB], [BW, TR], [512, C], [1, BW]],
                )
                r_bc = AP(
                    R[:].tensor, 0,
                    [[TR * BW, NB], [BW, TR], [0, C], [1, BW]],
                )
                nc.vector.tensor_tensor(
                    outst[:], p_view, r_bc, mybir.AluOpType.mult)
                st_eng[t % 2].dma_start(
                    out=d_out.ap()[:, t * TR:(t + 1) * TR],
                    in_=outst[:].rearrange("p r c x -> p r (c x)"),
                )

    nc.compile()
    return nc


def get_program():
    if "nc" not in _CACHE:
        _CACHE["nc"] = _build_program()
    return _CACHE["nc"]


def make_shards(data: np.ndarray, kernels: np.ndarray):
    """Full inputs -> per-core input maps (band layout, fp16)."""
    data = np.asarray(data, dtype=np.float32)
    kernels = np.asarray(kernels, dtype=np.float32)

    kf = kernels[:, TAP_PERM].astype(np.float16)      # [B, 25, H, W]
    dpad = np.zeros((B, C, H + 4, W + 6), dtype=np.float16)
    dpad[:, :, 2:H + 2, 2:W + 2] = data

    in_maps = []
    for core in range(NCORES):
        b, hh = divmod(core, 2)
        r0 = hh * HS
        ks = kf[b, :, r0:r0 + HS, :]                  # [25, 360, 1280]
        ke = np.ascontiguousarray(
            ks.reshape(TAPS, HS, NB, BW).transpose(2, 1, 0, 3)
        ).reshape(NB, HS, TAPS * BW)
        dsl = dpad[b, :, r0:r0 + DR, :]               # [3, 364, 1286]
        win = sliding_window_view(dsl, DW, axis=2)    # [3, 364, 1273, 14]
        db0 = np.ascontiguousarray(
            win[:, :, 0:NB * BW:BW].transpose(2, 0, 1, 3))
        db1 = np.ascontiguousarray(
            win[:, :, 1:NB * BW + 1:BW].transpose(2, 0, 1, 3))
        in_maps.append({"ke": ke, "db0": db0, "db1": db1})
    return in_maps


def unshard_out(arr: np.ndarray) -> np.ndarray:
    """Per-core out [NB, HS, C*BW] fp16 -> [C, HS, W] f32."""
    o = arr.reshape(NB, HS, C, BW).transpose(2, 1, 0, 3)
    return np.ascontiguousarray(o).reshape(C, HS, W).astype(np.float32)


def assemble(results) -> np.ndarray:
    out = np.empty((B, C, H, W), dtype=np.float32)
    for core in range(NCORES):
        b, hh = divmod(core, 2)
        out[b, :, hh * HS:(hh + 1) * HS, :] = unshard_out(results[core]["out"])
    return out


def kernel(data: np.ndarray, kernels: np.ndarray) -> np.ndarray:
    from concourse.bass_utils import run_bass_kernel_spmd

    nc = get_program()
    in_maps = make_shards(data, kernels)
    res = run_bass_kernel_spmd(nc, in_maps, list(range(NCORES)))
    return assemble(res.results)


if __name__ == "__main__":
    get_program()
    print("program built OK")


# revision 5
# speedup vs baseline: 1.5791x; 1.0974x over previous
"""Trainium2 Bass kernel: per-pixel 5x5 kernel application (KPN-style).

    out[b,c,y,x] = sum_{i,j} softmax(kernels[b,:,y,x])[i*5+j]
                   * zpad(data)[b,c,y+i,x+j]          (i,j in 0..4, r=2)

Sharding (8 NeuronCores, pure data parallel, no collectives):
    core = (b, H-half): 4 batches x 2 row-bands of 360 rows.

Band layout: partition p = x-band of 10 columns (128 bands x 10 = 1280).
Rows live in the free dimension, so BOTH the di (row) and dj (col) tap
shifts become free-dim AP offsets -- no shift matrices, no partition
crossing.  Per accumulation tile (15/30-row warmup tiles, then 45 rows):

    - ACT: E = exp(logits) in one op (fp16).
    - DVE: per (di, c): two batched products q = E * D (dj in the AP's
      outer free dim; even dj read D0, odd dj read D1 = D0 shifted one
      element so operands stay 4-byte aligned for the 2x DVE mode).
      A few odd-dj product ops run on GpSimd instead (tensor_tensor is
      2x_1P on DVE, so the two engines don't contend for SBUF ports).
    - PE:  identity-lhsT matmuls accumulate the 75 tap planes and the
      25 exp planes into 4 PSUM banks (start/stop per bank).  The
      stationary operand never changes, so the PE stays warm.
    - normalize: R32 = reciprocal_approx_fast(sumexp) (DVE), R16 cast
      (GpSimd), P copied PSUM->SBUF fp16 (ACT), out = P * R (DVE, 2x).

DMA: everything big rides SWDGE (gpsimd) so descriptors spray across
all 16 SDMA engines with one contiguous run per partition.  The first
kE load is emitted BEFORE the D tiles so nothing queues ahead of the
exp->product critical path at startup; D0/D1 are split in two row
chunks each.  Stores ride the HWDGE rings (engines 0-3, otherwise idle).

kernel(**inputs) takes the FULL inputs and returns the FULL output.
"""

import numpy as np
from numpy.lib.stride_tricks import sliding_window_view

B, C, H, W, KW = 4, 3, 720, 1280, 5
NCORES = 8
HS = H // 2            # 360 output rows per shard
NB = 128               # x-bands (partitions)
BW = 10                # band width (output columns per partition)
DW = 14                # data band width incl. 2+2 halo columns
DR = HS + 4            # 364 data rows incl. 2+2 halo rows
TAPS = KW * KW

# accumulation tiles: small warmup tiles shorten the startup pipeline
TILES = [(0, 15), (15, 30)] + [(45 * k, 45) for k in range(1, 8)]
DSPLIT = 94            # D tiles load in rows [0,94) + [94,364)

# host tap order: within each di group, dj = 0,2,4,1,3 (even-first so
# the even/odd product APs are plain slices)
DJ_ORDER = [0, 2, 4, 1, 3]
TAP_PERM = [di * KW + dj for di in range(KW) for dj in DJ_ORDER]

# odd-dj product ops computed on GpSimd instead of DVE
# (empty: gpsimd tensor ops hung the exec unit on HW - see notes)
GP_QO = set()
GP_RCAST = False

_CACHE = {}


def _build_program():
    import concourse.bacc as bacc
    import concourse.mybir as mybir
    from concourse.bass import AP
    from concourse import tile

    f32 = mybir.dt.float32
    f16 = mybir.dt.float16

    nc = bacc.Bacc(
        "TRN2",
        target_bir_lowering=False,
        debug=False,
        enable_asserts=False,
        num_devices=NCORES,
    )
    d_ke = nc.dram_tensor("ke", [NB, HS, TAPS * BW], f16, kind="ExternalInput")
    d_db0 = nc.dram_tensor("db0", [NB, DR, C, DW], f16, kind="ExternalInput")
    d_db1 = nc.dram_tensor("db1", [NB, DR, C, DW], f16, kind="ExternalInput")
    d_out = nc.dram_tensor("out", [NB, HS, C * BW], f16, kind="ExternalOutput")

    d_id = nc.inline_tensor(np.eye(NB, dtype=np.float16), "ident")

    KE_ROW = TAPS * BW          # 250 elems per (band, row)
    D_ROW = C * DW              # 42 elems per (band, row)

    with tile.TileContext(nc) as tc:
        with tc.tile_pool(name="const", bufs=1) as cpool, \
             tc.tile_pool(name="kt", bufs=2) as kpool, \
             tc.tile_pool(name="et", bufs=2) as epool, \
             tc.tile_pool(name="dt", bufs=1) as dpool, \
             tc.tile_pool(name="qt", bufs=4) as qpool, \
             tc.tile_pool(name="rt", bufs=2) as rpool, \
             tc.tile_pool(name="ot", bufs=2) as opool, \
             tc.tile_pool(name="ps", bufs=2, space="PSUM") as ppool:

            id_sb = cpool.tile([NB, NB], f16)
            nc.sync.dma_start(out=id_sb[:], in_=d_id.ap())

            D0 = dpool.tile([NB, DR, C, DW], f16, tag="d0")
            D1 = dpool.tile([NB, DR, C, DW], f16, tag="d1")

            st_eng = [nc.sync, nc.scalar]

            for t, (r0, nr) in enumerate(TILES):
                fd = nr * BW
                kE = kpool.tile([NB, nr, KE_ROW], f16, tag="ke")
                nc.gpsimd.dma_start(
                    out=kE[:], in_=d_ke.ap()[:, r0:r0 + nr]
                )
                # D chunks ride the same SWDGE queue, behind the kE
                # loads whose consumers they race
                if t == 0:
                    nc.gpsimd.dma_start(
                        out=D0[:, 0:DSPLIT], in_=d_db0.ap()[:, 0:DSPLIT])
                    nc.gpsimd.dma_start(
                        out=D1[:, 0:DSPLIT], in_=d_db1.ap()[:, 0:DSPLIT])
                elif t == 1:
                    nc.gpsimd.dma_start(
                        out=D0[:, DSPLIT:DR], in_=d_db0.ap()[:, DSPLIT:DR])
                    nc.gpsimd.dma_start(
                        out=D1[:, DSPLIT:DR], in_=d_db1.ap()[:, DSPLIT:DR])

                E = epool.tile([NB, nr, TAPS, BW], f16, tag="e")
                nc.scalar.activation(
                    E[:].rearrange("p r t x -> p (r t x)"),
                    kE[:].rearrange("p r k -> p (r k)"),
                    mybir.ActivationFunctionType.Exp,
                )
                eap = E[:]

                pacc = ppool.tile([NB, 4, 512], f32, tag="pacc")

                for di in range(KW):
                    # products: q = E * D, dj batched in the outer free dim
                    qes, qos = [], []
                    for c in range(C):
                        qe = qpool.tile([NB, 3, nr, BW], f16, tag="qe")
                        qo = qpool.tile([NB, 2, nr, BW], f16, tag="qo")
                        e_even = AP(
                            eap.tensor, (KW * di) * BW,
                            [[nr * KE_ROW, NB], [BW, 3], [KE_ROW, nr], [1, BW]],
                        )
                        e_odd = AP(
                            eap.tensor, (KW * di + 3) * BW,
                            [[nr * KE_ROW, NB], [BW, 2], [KE_ROW, nr], [1, BW]],
                        )
                        doff = (r0 + di) * D_ROW + c * DW
                        d_even = AP(
                            D0[:].tensor, doff,
                            [[DR * D_ROW, NB], [2, 3], [D_ROW, nr], [1, BW]],
                        )
                        d_odd = AP(
                            D1[:].tensor, doff,
                            [[DR * D_ROW, NB], [2, 2], [D_ROW, nr], [1, BW]],
                        )
                        nc.vector.tensor_tensor(
                            qe[:], e_even, d_even, mybir.AluOpType.mult)
                        qo_eng = nc.gpsimd if (di, c) in GP_QO else nc.vector
                        qo_eng.tensor_tensor(
                            qo[:], e_odd, d_odd, mybir.AluOpType.mult)
                        qes.append(qe)
                        qos.append(qo)
                    # sumexp: 5 identity matmuls straight off E (no DVE dep)
                    for k in range(KW):
                        tp = KW * di + k
                        nc.tensor.matmul(
                            out=pacc[:, 3, 0:fd],
                            lhsT=id_sb[:],
                            rhs=eap[:, :, tp, :],
                            start=(tp == 0),
                            stop=(tp == TAPS - 1),
                        )
                    # tap accumulation
                    for c in range(C):
                        for k in range(3):
                            nc.tensor.matmul(
                                out=pacc[:, c, 0:fd],
                                lhsT=id_sb[:],
                                rhs=qes[c][:, k],
                                start=(di == 0 and k == 0),
                                stop=False,
                            )
                        for k in range(2):
                            nc.tensor.matmul(
                                out=pacc[:, c, 0:fd],
                                lhsT=id_sb[:],
                                rhs=qos[c][:, k],
                                start=False,
                                stop=(di == KW - 1 and k == 1),
                            )

                R32 = rpool.tile([NB, nr, BW], f32, tag="r32")
                nc.vector.reciprocal_approx_fast(
                    out=R32[:].rearrange("p r x -> p (r x)"),
                    in_=pacc[:, 3, 0:fd],
                )
                R16 = rpool.tile([NB, nr, BW], f16, tag="r16")
                if GP_RCAST:
                    nc.gpsimd.tensor_scalar_mul(
                        R16[:].rearrange("p r x -> p (r x)"),
                        R32[:].rearrange("p r x -> p (r x)"),
                        1.0,
                    )
                else:
                    nc.vector.tensor_copy(
                        R16[:].rearrange("p r x -> p (r x)"),
                        R32[:].rearrange("p r x -> p (r x)"),
                    )
                # P: PSUM -> SBUF fp16 on ACT (frees DVE from the slow
                # 1x PSUM-source read)
                Pst = opool.tile([NB, nr, C, BW], f16, tag="pst")
                p_view = AP(
                    pacc[:].tensor, 0,
                    [[4 * 512, NB], [BW, nr], [512, C], [1, BW]],
                )
                nc.scalar.activation(
                    Pst[:], p_view, mybir.ActivationFunctionType.Copy,
                )
                outst = opool.tile([NB, nr, C, BW], f16, tag="o")
                r_bc = AP(
                    R16[:].tensor, 0,
                    [[nr * BW, NB], [BW, nr], [0, C], [1, BW]],
                )
                nc.vector.tensor_tensor(
                    outst[:], Pst[:], r_bc, mybir.AluOpType.mult)
                st_eng[t % 2].dma_start(
                    out=d_out.ap()[:, r0:r0 + nr],
                    in_=outst[:].rearrange("p r c x -> p r (c x)"),
                )

    nc.compile()
    return nc


def get_program():
    if "nc" not in _CACHE:
        _CACHE["nc"] = _build_program()
    return _CACHE["nc"]


def make_shards(data: np.ndarray, kernels: np.ndarray):
    """Full inputs -> per-core input maps (band layout, fp16)."""
    data = np.asarray(data, dtype=np.float32)
    kernels = np.asarray(kernels, dtype=np.float32)

    kf = kernels[:, TAP_PERM].astype(np.float16)      # [B, 25, H, W]
    dpad = np.zeros((B, C, H + 4, W + 6), dtype=np.float16)
    dpad[:, :, 2:H + 2, 2:W + 2] = data

    in_maps = []
    for core in range(NCORES):
        b, hh = divmod(core, 2)
        r0 = hh * HS
        ks = kf[b, :, r0:r0 + HS, :]                  # [25, 360, 1280]
        ke = np.ascontiguousarray(
            ks.reshape(TAPS, HS, NB, BW).transpose(2, 1, 0, 3)
        ).reshape(NB, HS, TAPS * BW)
        dsl = dpad[b, :, r0:r0 + DR, :]               # [3, 364, 1286]
        win = sliding_window_view(dsl, DW, axis=2)    # [3, 364, 1273, 14]
        db0 = np.ascontiguousarray(
            win[:, :, 0:NB * BW:BW].transpose(2, 1, 0, 3))   # [128,364,3,14]
        db1 = np.ascontiguousarray(
            win[:, :, 1:NB * BW + 1:BW].transpose(2, 1, 0, 3))
        in_maps.append({"ke": ke, "db0": db0, "db1": db1})
    return in_maps


def unshard_out(arr: np.ndarray) -> np.ndarray:
    """Per-core out [NB, HS, C*BW] fp16 -> [C, HS, W] f32."""
    o = arr.reshape(NB, HS, C, BW).transpose(2, 1, 0, 3)
    return np.ascontiguousarray(o).reshape(C, HS, W).astype(np.float32)


def assemble(results) -> np.ndarray:
    out = np.empty((B, C, H, W), dtype=np.float32)
    for core in range(NCORES):
        b, hh = divmod(core, 2)
        out[b, :, hh * HS:(hh + 1) * HS, :] = unshard_out(results[core]["out"])
    return out


def kernel(data: np.ndarray, kernels: np.ndarray) -> np.ndarray:
    from concourse.bass_utils import run_bass_kernel_spmd

    nc = get_program()
    in_maps = make_shards(data, kernels)
    res = run_bass_kernel_spmd(nc, in_maps, list(range(NCORES)))
    return assemble(res.results)


if __name__ == "__main__":
    get_program()
    print("program built OK")


# revision 7
# speedup vs baseline: 1.6492x; 1.0444x over previous
"""Trainium2 Bass kernel: per-pixel 5x5 kernel application (KPN-style).

    out[b,c,y,x] = sum_{i,j} softmax(kernels[b,:,y,x])[i*5+j]
                   * zpad(data)[b,c,y+i,x+j]          (i,j in 0..4, r=2)

Sharding (8 NeuronCores, pure data parallel, no collectives):
    core = (b, H-half): 4 batches x 2 row-bands of 360 rows.

Band layout: partition p = x-band of 10 columns (128 bands x 10 = 1280).
Rows live in the free dimension, so BOTH the di (row) and dj (col) tap
shifts become free-dim AP offsets -- no shift matrices, no partition
crossing.  Per accumulation tile (15/30-row warmup tiles, then 45 rows):

    - ACT: E = exp(logits) in one op (fp16).
    - DVE: per (di, c): two batched products q = E * D (dj in the AP's
      outer free dim; even dj read D0, odd dj read D1 = D0 shifted one
      element so operands stay 4-byte aligned for the 2x DVE mode).
      A few odd-dj product ops run on GpSimd instead (tensor_tensor is
      2x_1P on DVE, so the two engines don't contend for SBUF ports).
    - PE:  identity-lhsT matmuls accumulate the 75 tap planes and the
      25 exp planes into 4 PSUM banks (start/stop per bank).  The
      stationary operand never changes, so the PE stays warm.
    - normalize: R32 = reciprocal_approx_fast(sumexp) (DVE), R16 cast
      (GpSimd), P copied PSUM->SBUF fp16 (ACT), out = P * R (DVE, 2x).

DMA: everything big rides SWDGE (gpsimd) so descriptors spray across
all 16 SDMA engines with one contiguous run per partition.  The first
kE load is emitted BEFORE the D tiles so nothing queues ahead of the
exp->product critical path at startup; D0/D1 are split in two row
chunks each.  Stores ride the HWDGE rings (engines 0-3, otherwise idle).

kernel(**inputs) takes the FULL inputs and returns the FULL output.
"""

import numpy as np
from numpy.lib.stride_tricks import sliding_window_view

B, C, H, W, KW = 4, 3, 720, 1280, 5
NCORES = 8
HS = H // 2            # 360 output rows per shard
NB = 128               # x-bands (partitions)
BW = 10                # band width (output columns per partition)
DW = 14                # data band width incl. 2+2 halo columns
DR = HS + 4            # 364 data rows incl. 2+2 halo rows
TAPS = KW * KW

# accumulation tiles: small warmup tiles shorten the startup pipeline,
# a small final tile shortens the drain
TILES = ([(0, 15), (15, 30)] + [(45 * k, 45) for k in range(1, 7)]
         + [(315, 30), (345, 15)])
DSPLIT = 188           # D tiles load in rows [0,188) + [188,364)

# host tap order: within each di group, dj = 0,2,4,1,3 (even-first so
# the even/odd product APs are plain slices)
DJ_ORDER = [0, 2, 4, 1, 3]
TAP_PERM = [di * KW + dj for di in range(KW) for dj in DJ_ORDER]

# odd-dj product ops computed on GpSimd instead of DVE
# (empty: gpsimd tensor ops hung the exec unit on HW - see notes)
GP_QO = set()
GP_RCAST = False

_CACHE = {}


def _build_program():
    import concourse.bacc as bacc
    import concourse.mybir as mybir
    from concourse.bass import AP
    from concourse import tile

    f32 = mybir.dt.float32
    f16 = mybir.dt.float16

    nc = bacc.Bacc(
        "TRN2",
        target_bir_lowering=False,
        debug=False,
        enable_asserts=False,
        num_devices=NCORES,
    )
    d_ke = nc.dram_tensor("ke", [NB, HS, TAPS * BW], f16, kind="ExternalInput")
    d_db0 = nc.dram_tensor("db0", [NB, DR, C, DW], f16, kind="ExternalInput")
    d_db1 = nc.dram_tensor("db1", [NB, DR, C, DW], f16, kind="ExternalInput")
    d_out = nc.dram_tensor("out", [NB, HS, C * BW], f16, kind="ExternalOutput")

    d_id = nc.inline_tensor(np.eye(NB, dtype=np.float16), "ident")

    KE_ROW = TAPS * BW          # 250 elems per (band, row)
    D_ROW = C * DW              # 42 elems per (band, row)

    with tile.TileContext(nc) as tc:
        with tc.tile_pool(name="const", bufs=1) as cpool, \
             tc.tile_pool(name="kt", bufs=2) as kpool, \
             tc.tile_pool(name="et", bufs=2) as epool, \
             tc.tile_pool(name="dt", bufs=1) as dpool, \
             tc.tile_pool(name="qt", bufs=4) as qpool, \
             tc.tile_pool(name="rt", bufs=2) as rpool, \
             tc.tile_pool(name="ot", bufs=2) as opool, \
             tc.tile_pool(name="ps", bufs=2, space="PSUM") as ppool:

            id_sb = cpool.tile([NB, NB], f16)
            nc.sync.dma_start(out=id_sb[:], in_=d_id.ap())

            D0 = dpool.tile([NB, DR, C, DW], f16, tag="d0")
            D1 = dpool.tile([NB, DR, C, DW], f16, tag="d1")

            st_eng = [nc.sync, nc.scalar]

            for t, (r0, nr) in enumerate(TILES):
                fd = nr * BW
                kE = kpool.tile([NB, nr, KE_ROW], f16, tag="ke")
                nc.gpsimd.dma_start(
                    out=kE[:], in_=d_ke.ap()[:, r0:r0 + nr]
                )
                # D chunks ride the same SWDGE queue, behind the kE
                # loads whose consumers they race
                if t == 0:
                    nc.gpsimd.dma_start(
                        out=D0[:, 0:DSPLIT], in_=d_db0.ap()[:, 0:DSPLIT])
                    nc.gpsimd.dma_start(
                        out=D1[:, 0:DSPLIT], in_=d_db1.ap()[:, 0:DSPLIT])
                elif t == 2:
                    # rows >= 188 are first needed by tile 4; keep these
                    # big chunks behind tile 2's kE load in the SWDGE queue
                    nc.gpsimd.dma_start(
                        out=D0[:, DSPLIT:DR], in_=d_db0.ap()[:, DSPLIT:DR])
                    nc.gpsimd.dma_start(
                        out=D1[:, DSPLIT:DR], in_=d_db1.ap()[:, DSPLIT:DR])

                E = epool.tile([NB, nr, TAPS, BW], f16, tag="e")
                nc.scalar.activation(
                    E[:].rearrange("p r t x -> p (r t x)"),
                    kE[:].rearrange("p r k -> p (r k)"),
                    mybir.ActivationFunctionType.Exp,
                )
                eap = E[:]

                pacc = ppool.tile([NB, 4, 512], f32, tag="pacc")

                for di in range(KW):
                    # products: q = E * D, dj batched in the outer free dim
                    qes, qos = [], []
                    for c in range(C):
                        qe = qpool.tile([NB, 3, nr, BW], f16, tag="qe")
                        qo = qpool.tile([NB, 2, nr, BW], f16, tag="qo")
                        e_even = AP(
                            eap.tensor, (KW * di) * BW,
                            [[nr * KE_ROW, NB], [BW, 3], [KE_ROW, nr], [1, BW]],
                        )
                        e_odd = AP(
                            eap.tensor, (KW * di + 3) * BW,
                            [[nr * KE_ROW, NB], [BW, 2], [KE_ROW, nr], [1, BW]],
                        )
                        doff = (r0 + di) * D_ROW + c * DW
                        d_even = AP(
                            D0[:].tensor, doff,
                            [[DR * D_ROW, NB], [2, 3], [D_ROW, nr], [1, BW]],
                        )
                        d_odd = AP(
                            D1[:].tensor, doff,
                            [[DR * D_ROW, NB], [2, 2], [D_ROW, nr], [1, BW]],
                        )
                        nc.vector.tensor_tensor(
                            qe[:], e_even, d_even, mybir.AluOpType.mult)
                        qo_eng = nc.gpsimd if (di, c) in GP_QO else nc.vector
                        qo_eng.tensor_tensor(
                            qo[:], e_odd, d_odd, mybir.AluOpType.mult)
                        qes.append(qe)
                        qos.append(qo)
                    # sumexp: 5 identity matmuls straight off E (no DVE dep)
                    for k in range(KW):
                        tp = KW * di + k
                        nc.tensor.matmul(
                            out=pacc[:, 3, 0:fd],
                            lhsT=id_sb[:],
                            rhs=eap[:, :, tp, :],
                            start=(tp == 0),
                            stop=(tp == TAPS - 1),
                        )
                    # tap accumulation
                    for c in range(C):
                        for k in range(3):
                            nc.tensor.matmul(
                                out=pacc[:, c, 0:fd],
                                lhsT=id_sb[:],
                                rhs=qes[c][:, k],
                                start=(di == 0 and k == 0),
                                stop=False,
                            )
                        for k in range(2):
                            nc.tensor.matmul(
                                out=pacc[:, c, 0:fd],
                                lhsT=id_sb[:],
                                rhs=qos[c][:, k],
                                start=False,
                                stop=(di == KW - 1 and k == 1),
                            )

                R32 = rpool.tile([NB, nr, BW], f32, tag="r32")
                nc.vector.reciprocal_approx_fast(
                    out=R32[:].rearrange("p r x -> p (r x)"),
                    in_=pacc[:, 3, 0:fd],
                )
                R16 = rpool.tile([NB, nr, BW], f16, tag="r16")
                if GP_RCAST:
                    nc.gpsimd.tensor_scalar_mul(
                        R16[:].rearrange("p r x -> p (r x)"),
                        R32[:].rearrange("p r x -> p (r x)"),
                        1.0,
                    )
                else:
                    nc.vector.tensor_copy(
                        R16[:].rearrange("p r x -> p (r x)"),
                        R32[:].rearrange("p r x -> p (r x)"),
                    )
                # P: PSUM -> SBUF fp16 on ACT (frees DVE from the slow
                # 1x PSUM-source read)
                Pst = opool.tile([NB, nr, C, BW], f16, tag="pst")
                p_view = AP(
                    pacc[:].tensor, 0,
                    [[4 * 512, NB], [BW, nr], [512, C], [1, BW]],
                )
                nc.scalar.activation(
                    Pst[:], p_view, mybir.ActivationFunctionType.Copy,
                )
                outst = opool.tile([NB, nr, C, BW], f16, tag="o")
                r_bc = AP(
                    R16[:].tensor, 0,
                    [[nr * BW, NB], [BW, nr], [0, C], [1, BW]],
                )
                nc.vector.tensor_tensor(
                    outst[:], Pst[:], r_bc, mybir.AluOpType.mult)
                st_eng[t % 2].dma_start(
                    out=d_out.ap()[:, r0:r0 + nr],
                    in_=outst[:].rearrange("p r c x -> p r (c x)"),
                )

    nc.compile()
    return nc


def get_program():
    if "nc" not in _CACHE:
        _CACHE["nc"] = _build_program()
    return _CACHE["nc"]


def make_shards(data: np.ndarray, kernels: np.ndarray):
    """Full inputs -> per-core input maps (band layout, fp16)."""
    data = np.asarray(data, dtype=np.float32)
    kernels = np.asarray(kernels, dtype=np.float32)

    kf = kernels[:, TAP_PERM].astype(np.float16)      # [B, 25, H, W]
    dpad = np.zeros((B, C, H + 4, W + 6), dtype=np.float16)
    dpad[:, :, 2:H + 2, 2:W + 2] = data

    in_maps = []
    for core in range(NCORES):
        b, hh = divmod(core, 2)
        r0 = hh * HS
        ks = kf[b, :, r0:r0 + HS, :]                  # [25, 360, 1280]
        ke = np.ascontiguousarray(
            ks.reshape(TAPS, HS, NB, BW).transpose(2, 1, 0, 3)
        ).reshape(NB, HS, TAPS * BW)
        dsl = dpad[b, :, r0:r0 + DR, :]               # [3, 364, 1286]
        win = sliding_window_view(dsl, DW, axis=2)    # [3, 364, 1273, 14]
        db0 = np.ascontiguousarray(
            win[:, :, 0:NB * BW:BW].transpose(2, 1, 0, 3))   # [128,364,3,14]
        db1 = np.ascontiguousarray(
            win[:, :, 1:NB * BW + 1:BW].transpose(2, 1, 0, 3))
        in_maps.append({"ke": ke, "db0": db0, "db1": db1})
    return in_maps


def unshard_out(arr: np.ndarray) -> np.ndarray:
    """Per-core out [NB, HS, C*BW] fp16 -> [C, HS, W] f32."""
    o = arr.reshape(NB, HS, C, BW).transpose(2, 1, 0, 3)
    return np.ascontiguousarray(o).reshape(C, HS, W).astype(np.float32)


def assemble(results) -> np.ndarray:
    out = np.empty((B, C, H, W), dtype=np.float32)
    for core in range(NCORES):
        b, hh = divmod(core, 2)
        out[b, :, hh * HS:(hh + 1) * HS, :] = unshard_out(results[core]["out"])
    return out


def kernel(data: np.ndarray, kernels: np.ndarray) -> np.ndarray:
    from concourse.bass_utils import run_bass_kernel_spmd

    nc = get_program()
    in_maps = make_shards(data, kernels)
    res = run_bass_kernel_spmd(nc, in_maps, list(range(NCORES)))
    return assemble(res.results)


if __name__ == "__main__":
    get_program()
    print("program built OK")
